# revision 44
# baseline (speedup 1.0000x reference)
"""Multi-head attention (RoPE, softmax, out-proj) on 8 Trainium2 NeuronCores.

Sharding: batch (2) x head-groups (4) -> 8 cores. Each core computes, for its
batch b and its 4 heads: q/k/v projections (column-parallel), RoPE, full
attention, and a partial output projection against its slice of wo
(row-parallel). The 4 partial outputs per batch are summed on the host.

Matmuls run in bf16 (full PE rate, FWL weight loads) with fp32 PSUM
accumulation. The softmax is computed unnormalized (exp without max
subtraction is safe: scores ~ N(0,1)); the denominator is a bf16 halving
tree on the DVE over the exp tiles followed by a fast-approx reciprocal.

Layout trick: weights are pre-transposed on the host so every matmul operand
is a natural [contraction-dim-major] DMA. Within each head, q/k feature rows
are permuted to (even pairs, odd pairs) so RoPE's interleaved pair structure
becomes a partition-block structure (rows 0:64 / 64:128); scores are
invariant to the (shared) permutation and v/wo stay unpermuted. The halves
swap needed by RoPE's cross terms is done with two SBUF->SBUF DMAs on the
(otherwise idle) gpsimd queue and the signs are folded into the
(host-prepared) sin rows [+sin; -sin].

Stage B is software-pipelined per (query-chunk, head) unit: the PE issues the
NEXT unit's score matmuls before the current unit's PV matmuls, so the ACT
exp chain never starves; exp runs on [128,1024] PSUM pairs to amortize the
per-ACTIVATE fixed cost. The out-projection for the previous query chunk is
interleaved one row-slice per unit; its PSUM->SBUF bf16 cast runs on gpsimd
and the store DMA on the sync queue.
"""
import math
import sys

import numpy as np

for _p in ('/opt/trn_rl_repo', '/root/.axon_site/_ro/trn_rl_repo'):
    if _p not in sys.path:
        sys.path.insert(0, _p)

import ml_dtypes
import orjson

import concourse.bass as bass
import concourse.mybir as mybir
from concourse.tile import TileContext
from concourse.bass_utils import run_bass_kernel_spmd

F32 = mybir.dt.float32
BF16 = mybir.dt.bfloat16
NP_BF16 = ml_dtypes.bfloat16

B = 2
S = 2048
D = 2048
HD = 128
N_CORES = 8
GROUPS = 4          # head groups (tensor-parallel degree per batch)
HPC = (D // HD) // GROUPS  # heads per core (4)
LF = HPC * HD       # local features per core (512)


# ---------------------------------------------------------------------------
# Wait-splitting post-pass: this toolchain's walrus supports at most ONE sync
# wait command per instruction (none at all on fp32/fp32r Matmult, which
# lowers to an LDW+MM pair). Tile emits multi-wait instructions; hoist the
# excess onto NoOps on the same engine immediately before the instruction.
# ---------------------------------------------------------------------------

def _keep_count(ins):
    if ins.get('opcode') == 'Matmult':
        dt = None
        for arg in ins.get('ins', []):
            dt = arg.get('dtype') or dt
        if dt in ('float32', 'float32r'):
            return 0
        return 1
    if ins.get('opcode') == 'ISA':
        # custom-DVE ISA instructions have a fixed encoding with no room
        # for a sync wait command
        return 0
    return 1


def _split_waits_json(data: bytes) -> bytes:
    d = orjson.loads(data)
    ctr = 0
    for fn in d.get('functions', []):
        for bb in fn.get('blocks', []):
            out = []
            for ins in bb.get('instructions', []):
                si = ins.get('sync_info')
                waits = (si or {}).get('on_wait') or []
                keep = _keep_count(ins)
                if len(waits) > keep:
                    hoist = waits[:len(waits) - keep]
                    keep_w = waits[len(waits) - keep:]
                    for w in hoist:
                        ctr += 1
                        nop = {
                            'name': f"{ins['name']}-ws{ctr}",
                            'opcode': 'NoOp',
                            'engine': ins.get('engine'),
                            'ins': [],
                            'outs': [],
                            'sync_info': {'on_wait': [w], 'on_update': []},
                        }
                        if 'debug' in ins:
                            nop['debug'] = ins['debug']
                        out.append(nop)
                    si['on_wait'] = keep_w
                out.append(ins)
            bb['instructions'] = out
    return orjson.dumps(d)


def _install_waitsplit():
    if getattr(bass.Bass, '_waitsplit_installed', False):
        return
    orig = bass.Bass.to_json_bytes

    def patched(self, *a, **k):
        return _split_waits_json(orig(self, *a, **k))

    bass.Bass.to_json_bytes = patched
    bass.Bass._waitsplit_installed = True


_install_waitsplit()


# ---------------------------------------------------------------------------
# Device program (SPMD, identical on all cores; per-core data differs)
# ---------------------------------------------------------------------------

def build_nc(s=S, d=D, hpc=HPC):
    lf = hpc * HD
    kd_n = d // 128          # contraction chunks for projections
    nw = 512 if s >= 512 else s  # free-dim width per matmul
    nsq = s // nw            # wide column chunks
    ns = s // 128            # 128-row chunks (key chunks)
    nj = d // 512 if d >= 512 else 1
    jw = 512 if d >= 512 else d
    scale = 1.0 / math.sqrt(HD)
    npairs = ns // 2
    nunits = nsq * hpc
    nsub = nw // 128

    # All DRAM tensors are host-packed into SBUF layout ([128, ...] with the
    # kd/row blocks along the free dim) so every DMA moves >=4KB contiguous
    # per partition — 1KB-segment DMAs are descriptor-dominated (~30% of
    # peak) and were the startup bottleneck. y is packed the same way and
    # unpacked on the host.
    nc = bass.Bass()
    xT = nc.dram_tensor("xT", [128, kd_n * s], BF16, kind="ExternalInput")
    wqT = nc.dram_tensor("wqT", [128, kd_n * lf], BF16, kind="ExternalInput")
    wkT = nc.dram_tensor("wkT", [128, kd_n * lf], BF16, kind="ExternalInput")
    wvT = nc.dram_tensor("wvT", [128, kd_n * lf], BF16, kind="ExternalInput")
    woT = nc.dram_tensor("woT", [128, hpc * d], BF16, kind="ExternalInput")
    csd = nc.dram_tensor("csd", [128, s], BF16, kind="ExternalInput")
    snd = nc.dram_tensor("snd", [128, s], BF16, kind="ExternalInput")
    y = nc.dram_tensor("y", [128, (s // 128) * d], BF16, kind="ExternalOutput")
    # weight loads in blocks of up to 4 kd-chunks: big enough for efficient
    # DMA, small enough that the first matmuls start early
    wblk = min(4, kd_n)
    nwblk = kd_n // wblk

    with TileContext(nc) as tc:
        # Persistent SBUF residents: post-RoPE q/k (head-major), v (s-chunk
        # blocks), and the warm-up operand.
        with tc.tile_pool(name="persist", bufs=1) as per:
            qT_all = per.tile([128, hpc * s], BF16, name="qT_all")
            kT_all = per.tile([128, hpc * s], BF16, name="kT_all")
            v_all = per.tile([128, ns * lf], BF16, name="v_all")
            ones_b = per.tile([128, 128], BF16, name="ones_b")
            nc.vector.memset(ones_b, 1.0)

            # ---------- Stage A: q/k/v projections + RoPE (x streamed once) ----------
            with tc.tile_pool(name="wqk", bufs=1) as wpool, \
                 tc.tile_pool(name="csp", bufs=1) as cspool, \
                 tc.tile_pool(name="rp", bufs=2) as rpool, \
                 tc.tile_pool(name="psQ", bufs=5, space="PSUM") as psq_pool, \
                 tc.tile_pool(name="psW", bufs=1, space="PSUM") as pswarm, \
                 tc.tile_pool(name="psV", bufs=2, space="PSUM") as psv_pool:
                # PE clock warm-up: the startup is HBM-bound, so dummy
                # matmuls keep the HAM activity window non-idle (clock at
                # full rate) until the real matmuls flow. An initial batch
                # covers engine init; more are sprinkled through the first
                # chunk's DMA-paced phase below.
                wps = pswarm.tile([128, 128], F32, name="wps")
                for _ in range(48):
                    nc.tensor.matmul(wps, ones_b, ones_b, start=True, stop=True)
                wq_sb = wpool.tile([128, kd_n * lf], BF16, name="wq_sb")
                wk_sb = wpool.tile([128, kd_n * lf], BF16, name="wk_sb")
                wv_sb = wpool.tile([128, kd_n * lf], BF16, name="wv_sb")
                x_all = wpool.tile([128, kd_n * s], BF16, name="x_all")

                # cos/sin ride the gpsimd (SWDGE) queue so they don't delay
                # the wq/x stream on the sync queue
                cs_sb = cspool.tile([128, s], BF16, name="cs_sb")
                sn_sb = cspool.tile([128, s], BF16, name="sn_sb")
                nc.gpsimd.dma_start(out=cs_sb, in_=csd[:, :])
                nc.gpsimd.dma_start(out=sn_sb, in_=snd[:, :])
                # load order = consumption order. x is packed sq-outermost,
                # so each query chunk is one fat contiguous DMA; chunk 0 is
                # split in half so the first matmuls start a bit earlier.
                # sync queue: wq blocks + x chunks (the critical path);
                # scalar queue: cos/sin then wk; wv deferred past chunk 0.
                xw = kd_n * nw  # columns of one packed x chunk
                nc.sync.dma_start(out=wq_sb[:, 0:wblk * lf],
                                  in_=wqT[:, 0:wblk * lf])
                nc.sync.dma_start(out=x_all[:, 0:xw // 2],
                                  in_=xT[:, 0:xw // 2])
                if kd_n > wblk:
                    nc.sync.dma_start(out=wq_sb[:, wblk * lf:2 * wblk * lf],
                                      in_=wqT[:, wblk * lf:2 * wblk * lf])
                nc.sync.dma_start(out=x_all[:, xw // 2:xw],
                                  in_=xT[:, xw // 2:xw])
                for b0 in range(2 * wblk, kd_n, wblk):
                    nc.sync.dma_start(
                        out=wq_sb[:, b0 * lf:(b0 + wblk) * lf],
                        in_=wqT[:, b0 * lf:(b0 + wblk) * lf])
                for sq in range(1, nsq):
                    nc.sync.dma_start(out=x_all[:, sq * xw:(sq + 1) * xw],
                                      in_=xT[:, sq * xw:(sq + 1) * xw])
                for b0 in range(0, kd_n, wblk):
                    nc.scalar.dma_start(
                        out=wk_sb[:, b0 * lf:(b0 + wblk) * lf],
                        in_=wkT[:, b0 * lf:(b0 + wblk) * lf])

                def emit_v(sq):
                    # v for chunk sq, pipelined one chunk behind q/k: wv is the
                    # last weight to arrive and v isn't needed until stage B
                    for ss in range(nw // 128):
                        psv = psv_pool.tile([128, lf], F32, name="psv")
                        for kd in range(kd_n):
                            nc.tensor.matmul(
                                psv,
                                x_all[:, (sq * kd_n + kd) * nw + ss * 128:
                                      (sq * kd_n + kd) * nw + (ss + 1) * 128],
                                wv_sb[:, kd * lf:(kd + 1) * lf],
                                start=(kd == 0), stop=(kd == kd_n - 1))
                        nc.vector.tensor_copy(
                            v_all[:, (sq * (nw // 128) + ss) * lf:
                                  (sq * (nw // 128) + ss + 1) * lf], psv)

                for sq in range(nsq):
                    # k before q on the last chunk so stage B's first scores
                    # (which need ALL of k but only chunk 0 of q) start sooner
                    phases = ((wq_sb, qT_all), (wk_sb, kT_all))
                    if sq == nsq - 1:
                        phases = (phases[1], phases[0])
                    for wsb, dstT in phases:
                        # kd-major accumulation into per-head PSUM tiles: the
                        # first chunk's matmuls start as soon as each kd block
                        # of the weights/x arrives instead of after the whole
                        # tile
                        ps_h = [psq_pool.tile([128, nw], F32, name="ps_qk")
                                for _ in range(hpc)]
                        for kd in range(kd_n):
                            if sq == 0 and kd % wblk == 0:
                                # chunk 0 is DMA-paced: dummy matmuls between
                                # the weight-block boundaries keep the PE
                                # clock warm through the wait slivers
                                for _ in range(2):
                                    nc.tensor.matmul(wps, ones_b, ones_b,
                                                     start=True, stop=True)
                            for h in range(hpc):
                                nc.tensor.matmul(
                                    ps_h[h],
                                    wsb[:, kd * lf + h * 128: kd * lf + (h + 1) * 128],
                                    x_all[:, (sq * kd_n + kd) * nw:
                                          (sq * kd_n + kd + 1) * nw],
                                    start=(kd == 0), stop=(kd == kd_n - 1))
                        for h in range(hpc):
                            ps = ps_h[h]
                            tcc = rpool.tile([128, nw], F32, name="t_c")
                            tss = rpool.tile([128, nw], F32, name="t_s")
                            nc.vector.tensor_mul(tcc, ps, cs_sb[:, sq * nw:(sq + 1) * nw])
                            # sn_sb rows are [+sin; -sin]: after the half-swap the
                            # signed cross terms land with the right signs
                            nc.vector.tensor_mul(tss, ps, sn_sb[:, sq * nw:(sq + 1) * nw])
                            tsw = rpool.tile([128, nw], F32, name="t_sw")
                            nc.gpsimd.dma_start(out=tsw[0:64, :], in_=tss[64:128, :])
                            nc.gpsimd.dma_start(out=tsw[64:128, :], in_=tss[0:64, :])
                            nc.vector.tensor_add(
                                dstT[:, h * s + sq * nw: h * s + sq * nw + nw], tcc, tsw)
                    if sq == min(1, nsq - 1):
                        # wv after chunk 1: first needed by emit_v(0) at the
                        # end of chunk 1, far past the startup burst
                        for b0 in range(0, kd_n, wblk):
                            nc.scalar.dma_start(
                                out=wv_sb[:, b0 * lf:(b0 + wblk) * lf],
                                in_=wvT[:, b0 * lf:(b0 + wblk) * lf])
                    if sq > 0:
                        emit_v(sq - 1)
                emit_v(nsq - 1)

            # keep the PE busy (clock warm) across the stage boundary while
            # the last k-chunk RoPE drains on the DVE
            with tc.tile_pool(name="psW2", bufs=1, space="PSUM") as pswarm2:
                wps2 = pswarm2.tile([128, 128], F32, name="wps2")
                for _ in range(40):
                    nc.tensor.matmul(wps2, ones_b, ones_b, start=True, stop=True)

            # ---------- Stage B+C: attention, then out-proj per query chunk ----------
            with tc.tile_pool(name="exp", bufs=2) as expool, \
                 tc.tile_pool(name="fld", bufs=2) as fpool, \
                 tc.tile_pool(name="nrm", bufs=2) as npool, \
                 tc.tile_pool(name="atp", bufs=2) as atpool, \
                 tc.tile_pool(name="wop", bufs=1) as wopool, \
                 tc.tile_pool(name="yop", bufs=3) as yopool, \
                 tc.tile_pool(name="psS", bufs=2, space="PSUM") as pssc, \
                 tc.tile_pool(name="psO", bufs=2, space="PSUM") as psov, \
                 tc.tile_pool(name="psM", bufs=1, space="PSUM") as pssm, \
                 tc.tile_pool(name="psC", bufs=1, space="PSUM") as psc:
                wo_sb = wopool.tile([128, hpc * d], BF16, name="wo_sb")
                for i in range(hpc):
                    nc.scalar.dma_start(out=wo_sb[:, i * d:(i + 1) * d],
                                        in_=woT[:, i * d:(i + 1) * d])

                def scores_exp_pair(u, ex_tile, p):
                    # one [128,1024] PSUM pair of scores for unit u and its
                    # batched exp ACTIVATE (amortizes the ACT fixed cost)
                    sq, h = divmod(u, hpc)
                    qT_sl = qT_all[:, h * s + sq * nw: h * s + (sq + 1) * nw]
                    sps = pssc.tile([128, 2 * nw], F32, name="sps")
                    for half in (0, 1):
                        sk = 2 * p + half
                        nc.tensor.matmul(
                            sps[:, half * nw:(half + 1) * nw],
                            kT_all[:, h * s + sk * 128: h * s + (sk + 1) * 128],
                            qT_sl, start=True, stop=True)
                    nc.scalar.activation(
                        ex_tile[:, p * 2 * nw:(p + 1) * 2 * nw], sps,
                        mybir.ActivationFunctionType.Exp, scale=scale)

                def outproj_ops(psq, aT_tile, ssubs):
                    # out-projection micro-ops (one matmul or one cast each)
                    # for the given query-row slices of chunk psq; the caller
                    # interleaves them into PE wait slivers. The jn slices of
                    # one row-slice cast into a single [128, d] row tile which
                    # is stored with one wide (DMA-efficient) transfer.
                    ops = []
                    for ssub in ssubs:
                        yo_row = yopool.tile([128, d], BF16, name="yo_row")
                        for jn in range(nj):
                            yps = psc.tile([128, jw], F32, name="yps")
                            for i in range(hpc):
                                ops.append(
                                    lambda yps=yps, i=i, jn=jn, ssub=ssub:
                                    nc.tensor.matmul(
                                        yps,
                                        aT_tile[:, i * nw + ssub * 128:
                                                i * nw + (ssub + 1) * 128],
                                        wo_sb[:, i * d + jn * jw:
                                              i * d + (jn + 1) * jw],
                                        start=(i == 0), stop=(i == hpc - 1)))

                            def fin(yps=yps, jn=jn, ssub=ssub, yo_row=yo_row):
                                # split the PSUM->SBUF bf16 casts between ACT
                                # and DVE so neither becomes the bottleneck
                                if jn % 2 == 0:
                                    nc.scalar.copy(
                                        yo_row[:, jn * jw:(jn + 1) * jw], yps)
                                else:
                                    nc.vector.tensor_copy(
                                        yo_row[:, jn * jw:(jn + 1) * jw], yps)
                                if jn == nj - 1:
                                    row = psq * nsub + ssub
                                    nc.sync.dma_start(
                                        out=y[:, row * d:(row + 1) * d],
                                        in_=yo_row)
                            ops.append(fin)
                    return ops

                def fold_push(stack, ap, lvl):
                    # binary-counter combine: same depth-log2 rounding as a
                    # balanced tree, but each combine runs as soon as its two
                    # inputs exist — the last one lands right after the last
                    # exp instead of a full tree-latency later
                    while stack and stack[-1][1] == lvl:
                        prev, _ = stack.pop()
                        t = fpool.tile([128, 2 * nw], BF16, name=f"fold{lvl}")
                        nc.vector.tensor_add(t, prev, ap)
                        ap, lvl = t, lvl + 1
                    stack.append((ap, lvl))

                def fold_finish(stack):
                    ap, _ = stack.pop()
                    while stack:
                        prev, pl = stack.pop()
                        t = fpool.tile([128, 2 * nw], BF16, name=f"fold{pl}")
                        nc.vector.tensor_add(t, prev, ap)
                        ap = t
                    accb = npool.tile([128, nw], BF16, name="accb")
                    nc.vector.tensor_add(accb, ap[:, :nw], ap[:, nw:2 * nw])
                    return accb

                def emit_folds(ex_u):
                    # post-hoc variant (used only for unit 0's tile, emitted
                    # before the main loop)
                    stack = []
                    for p in range(npairs):
                        fold_push(stack, ex_u[:, p * 2 * nw:(p + 1) * 2 * nw], 0)
                    return fold_finish(stack)

                ex_tiles = {}
                ex_tiles[0] = expool.tile([128, ns * nw], BF16, name="ex_sb")
                for p in range(npairs):
                    scores_exp_pair(0, ex_tiles[0], p)
                accb_tiles = {0: emit_folds(ex_tiles[0])}
                prev_c = None  # (sq, aT_tile) of the previous chunk
                aT_sq = None
                for u in range(nunits):
                    sq, h = divmod(u, hpc)
                    if h == 0:
                        aT_sq = atpool.tile([128, hpc * nw], BF16, name="aT_sq")
                    # denominator part 2 first: by block start the fold tree
                    # for this unit is done (it ran during the previous
                    # block), so the ones-matmul (partition reduction +
                    # broadcast) and the reciprocal clear immediately and
                    # nothing downstream waits on the DVE late in the block.
                    # Exception: at u=0 the fold tree only starts with stage
                    # B, so it would head the PE queue and block everything.
                    def emit_sm_recip():
                        sm = pssm.tile([128, nw], F32, name="sm")
                        nc.tensor.matmul(sm, ones_b, accb_tiles.pop(u),
                                         start=True, stop=True)
                        rec = npool.tile([128, nw], F32, name="rec")
                        nc.vector.reciprocal(rec, sm)
                        return rec

                    if u > 0:
                        rec = emit_sm_recip()
                    # out-projection micro-ops of the PREVIOUS chunk for this
                    # unit's row slices, to be interleaved below
                    if prev_c is not None:
                        psq, pat = prev_c
                        ops = outproj_ops(
                            psq, pat,
                            range(h * nsub // hpc, (h + 1) * nsub // hpc))
                    else:
                        ops = []
                    # interleave per score-pair: the next unit's scores+exp
                    # (paced by the ACT chain via the PSUM ring), this unit's
                    # PV matmuls, and the out-projection micro-ops fill the
                    # PE slivers in between
                    if u + 1 < nunits:
                        ex_tiles[u + 1] = expool.tile([128, ns * nw], BF16,
                                                      name="ex_sb")
                    ex_u = ex_tiles.pop(u)
                    ov = psov.tile([128, nw], F32, name="ov")
                    oi = 0
                    fold_stack = []
                    for p in range(npairs):
                        if u + 1 < nunits:
                            scores_exp_pair(u + 1, ex_tiles[u + 1], p)
                            # progressive fold of the prefetched exp pairs:
                            # the denominator for unit u+1 is complete right
                            # after its last exp, so the next block's
                            # ones-matmul never stalls the PE queue
                            fold_push(
                                fold_stack,
                                ex_tiles[u + 1][:, p * 2 * nw:(p + 1) * 2 * nw],
                                0)
                        for half in (0, 1):
                            sk = 2 * p + half
                            nc.tensor.matmul(ov,
                                             v_all[:, sk * lf + h * 128:
                                                   sk * lf + (h + 1) * 128],
                                             ex_u[:, sk * nw:(sk + 1) * nw],
                                             start=(sk == 0),
                                             stop=(sk == ns - 1))
                        take = (((p + 1) * len(ops)) // npairs
                                - (p * len(ops)) // npairs)
                        for _ in range(take):
                            ops[oi]()
                            oi += 1
                        if not ops:
                            # first chunk has no out-projection yet: standalone
                            # weight loads keep the PE activity window non-idle
                            # (clock warm) through the ACT-paced wait slivers
                            for _ in range(2):
                                nc.tensor.ldweights(ones_b)
                    if u == 0:
                        rec = emit_sm_recip()
                    nc.vector.tensor_mul(aT_sq[:, h * nw:(h + 1) * nw], ov, rec)
                    if u + 1 < nunits:
                        accb_tiles[u + 1] = fold_finish(fold_stack)
                    if h == hpc - 1:
                        prev_c = (sq, aT_sq)
                # drain the final chunk's out-projection
                psq, pat = prev_c
                for op in outproj_ops(psq, pat, range(nsub)):
                    op()
    return nc


# ---------------------------------------------------------------------------
# Host-side sharding + gather
# ---------------------------------------------------------------------------

_PERM_HEAD = np.concatenate([np.arange(0, HD, 2), np.arange(1, HD, 2)])


def _pack_rows(a):
    """[n*128, m] -> [128, n*m]: kd-blocks of 128 rows side by side along the
    free dim — the SBUF-resident layout, so device DMAs are contiguous."""
    n = a.shape[0] // 128
    return np.ascontiguousarray(
        a.reshape(n, 128, a.shape[1]).transpose(1, 0, 2).reshape(128, -1))


def _unpack_y(yp, s, d):
    """[128, (s//128)*d] -> [s, d] (inverse of the device's packed store)."""
    n = s // 128
    return yp.reshape(128, n, d).transpose(1, 0, 2).reshape(s, d)


def _prep_in_maps(x, wq, wk, wv, wo, pos_cos, pos_sin, s=S, d=D, hpc=HPC):
    lf = hpc * HD
    h_total = d // HD
    groups = h_total // hpc
    # permute q/k feature rows within each head: even pairs first, then odd
    wq_p = wq.reshape(h_total, HD, d)[:, _PERM_HEAD, :].reshape(d, d)
    wk_p = wk.reshape(h_total, HD, d)[:, _PERM_HEAD, :].reshape(d, d)
    wqT_full = np.ascontiguousarray(wq_p.T).astype(NP_BF16)
    wkT_full = np.ascontiguousarray(wk_p.T).astype(NP_BF16)
    wvT_full = np.ascontiguousarray(wv.T).astype(NP_BF16)
    woT_full = np.ascontiguousarray(wo.T).astype(NP_BF16)
    cs_half = np.ascontiguousarray(pos_cos[0].T).astype(np.float32)  # [64, S]
    sn_half = np.ascontiguousarray(pos_sin[0].T).astype(np.float32)
    csd = np.concatenate([cs_half, cs_half], axis=0).astype(NP_BF16)
    snd = np.concatenate([sn_half, -sn_half], axis=0).astype(NP_BF16)
    in_maps = []
    n_batches = x.shape[0]
    # x packed sq-outermost: [128, sq][kd][nw] so each query chunk of every
    # contraction block is one contiguous device DMA
    kd_n = d // 128
    nw = 512 if s >= 512 else s
    nsq = s // nw

    def pack_x(xb):
        xt = np.ascontiguousarray(xb.T).astype(NP_BF16)  # [d, s]
        return np.ascontiguousarray(
            xt.reshape(kd_n, 128, nsq, nw).transpose(1, 2, 0, 3)
            .reshape(128, kd_n * s))

    xP = [pack_x(x[b]) for b in range(n_batches)]
    for c in range(n_batches * groups):
        b, g = divmod(c, groups)
        in_maps.append({
            "xT": xP[b],
            "wqT": _pack_rows(wqT_full[:, g * lf:(g + 1) * lf]),
            "wkT": _pack_rows(wkT_full[:, g * lf:(g + 1) * lf]),
            "wvT": _pack_rows(wvT_full[:, g * lf:(g + 1) * lf]),
            "woT": _pack_rows(woT_full[g * lf:(g + 1) * lf, :]),
            "csd": csd,
            "snd": snd,
        })
    return in_maps


_NC_CACHE = {}


def _get_nc(s=S, d=D, hpc=HPC):
    key = (s, d, hpc)
    if key not in _NC_CACHE:
        _NC_CACHE[key] = build_nc(s, d, hpc)
    return _NC_CACHE[key]


def _np_rope(t, cos, sin):
    b, ss, hh, hd = t.shape
    tr = t.reshape(b, ss, hh, hd // 2, 2)
    te, to = tr[..., 0], tr[..., 1]
    c = cos[:, :, None, :]
    s = sin[:, :, None, :]
    return np.stack([te * c - to * s, te * s + to * c], axis=-1).reshape(b, ss, hh, hd)


def _score_sample_max(x, wq, wk, pos_cos, pos_sin):
    """Sampled estimate of max |score|; the device softmax skips the max
    subtraction, which is only safe when scores stay well under exp's fp32
    range."""
    ss = x[:, :: max(1, x.shape[1] // 32), :][:, :32]
    pos_idx = np.arange(x.shape[1])[:: max(1, x.shape[1] // 32)][:32]
    h = x.shape[2] // HD
    q = (ss @ wq.T).reshape(ss.shape[0], -1, h, HD)
    k = (ss @ wk.T).reshape(ss.shape[0], -1, h, HD)
    c = pos_cos[:, pos_idx]
    sn = pos_sin[:, pos_idx]
    q = _np_rope(q, c, sn)
    k = _np_rope(k, c, sn)
    sc = np.einsum('bqhd,bkhd->bhqk', q, k) / math.sqrt(HD)
    return float(np.abs(sc).max())


def _np_fallback(x, wq, wk, wv, wo, pos_cos, pos_sin):
    out = np.empty_like(x)
    h = x.shape[2] // HD
    for b in range(x.shape[0]):
        q = _np_rope((x[b:b + 1] @ wq.T).reshape(1, -1, h, HD), pos_cos, pos_sin)
        k = _np_rope((x[b:b + 1] @ wk.T).reshape(1, -1, h, HD), pos_cos, pos_sin)
        v = (x[b:b + 1] @ wv.T).reshape(1, -1, h, HD)
        sc = np.einsum('bqhd,bkhd->bhqk', q, k) / math.sqrt(HD)
        sc -= sc.max(axis=-1, keepdims=True)
        e = np.exp(sc, dtype=np.float32)
        p = e / e.sum(axis=-1, keepdims=True)
        out[b] = (np.einsum('bhqk,bkhd->bqhd', p, v).reshape(1, x.shape[1], -1)
                  @ wo.T)[0]
    return out


def kernel(x, wq, wk, wv, wo, pos_cos, pos_sin):
    x = np.asarray(x, dtype=np.float32)
    wq, wk, wv, wo = (np.asarray(a, dtype=np.float32) for a in (wq, wk, wv, wo))
    pos_cos = np.asarray(pos_cos, dtype=np.float32)
    pos_sin = np.asarray(pos_sin, dtype=np.float32)
    # the device softmax skips max subtraction (safe for scores ~ N(0,1));
    # if the inputs are scaled such that exp would overflow, fall back to a
    # correct (slower) host path rather than returning inf/NaN
    if 4.0 * _score_sample_max(x, wq, wk, pos_cos, pos_sin) > 80.0:
        return _np_fallback(x, wq, wk, wv, wo, pos_cos, pos_sin)
    in_maps = _prep_in_maps(x, wq, wk, wv, wo, pos_cos, pos_sin)
    nc = _get_nc()
    res = run_bass_kernel_spmd(nc, in_maps, core_ids=list(range(N_CORES)))
    out = np.empty((B, S, D), dtype=np.float32)
    for b in range(B):
        acc = _unpack_y(res.results[b * GROUPS]["y"].astype(np.float32), S, D)
        for g in range(1, GROUPS):
            acc = acc + _unpack_y(
                res.results[b * GROUPS + g]["y"].astype(np.float32), S, D)
        out[b] = acc
    return out


# revision 52
# speedup vs baseline: 1.0966x; 1.0966x over previous
"""Multi-head attention (RoPE, softmax, out-proj) on 8 Trainium2 NeuronCores.

Sharding: batch (2) x head-groups (4) -> 8 cores. Each core computes, for its
batch b and its 4 heads: q/k/v projections (column-parallel), RoPE, full
attention, and a partial output projection against its slice of wo
(row-parallel). The 4 partial outputs per batch are summed on the host.

Matmuls run in bf16 (full PE rate, FWL weight loads) with fp32 PSUM
accumulation. The softmax is computed unnormalized (exp without max
subtraction is safe: scores ~ N(0,1)); the denominator is a bf16 halving
tree on the DVE over the exp tiles followed by a fast-approx reciprocal.

Layout trick: weights are pre-transposed on the host so every matmul operand
is a natural [contraction-dim-major] DMA. Within each head, q/k feature rows
are permuted to (even pairs, odd pairs) so RoPE's interleaved pair structure
becomes a partition-block structure (rows 0:64 / 64:128); scores are
invariant to the (shared) permutation and v/wo stay unpermuted. The halves
swap needed by RoPE's cross terms is done with two SBUF->SBUF DMAs on the
(otherwise idle) gpsimd queue and the signs are folded into the
(host-prepared) sin rows [+sin; -sin].

Stage B is software-pipelined per (query-chunk, head) unit: the PE issues the
NEXT unit's score matmuls before the current unit's PV matmuls, so the ACT
exp chain never starves; exp runs on [128,1024] PSUM pairs to amortize the
per-ACTIVATE fixed cost. The out-projection for the previous query chunk is
interleaved one row-slice per unit; its PSUM->SBUF bf16 cast runs on gpsimd
and the store DMA on the sync queue.
"""
import math
import sys

import numpy as np

for _p in ('/opt/trn_rl_repo', '/root/.axon_site/_ro/trn_rl_repo'):
    if _p not in sys.path:
        sys.path.insert(0, _p)

import ml_dtypes
import orjson

import concourse.bass as bass
import concourse.mybir as mybir
from concourse.tile import TileContext
from concourse.bass_utils import run_bass_kernel_spmd

F32 = mybir.dt.float32
BF16 = mybir.dt.bfloat16
NP_BF16 = ml_dtypes.bfloat16

B = 2
S = 2048
D = 2048
HD = 128
N_CORES = 8
GROUPS = 4          # head groups (tensor-parallel degree per batch)
HPC = (D // HD) // GROUPS  # heads per core (4)
LF = HPC * HD       # local features per core (512)


# ---------------------------------------------------------------------------
# Wait-splitting post-pass: this toolchain's walrus supports at most ONE sync
# wait command per instruction (none at all on fp32/fp32r Matmult, which
# lowers to an LDW+MM pair). Tile emits multi-wait instructions; hoist the
# excess onto NoOps on the same engine immediately before the instruction.
# ---------------------------------------------------------------------------

def _keep_count(ins):
    if ins.get('opcode') == 'Matmult':
        dt = None
        for arg in ins.get('ins', []):
            dt = arg.get('dtype') or dt
        if dt in ('float32', 'float32r'):
            return 0
        return 1
    if ins.get('opcode') == 'ISA':
        # custom-DVE ISA instructions have a fixed encoding with no room
        # for a sync wait command
        return 0
    return 1


def _split_waits_json(data: bytes) -> bytes:
    d = orjson.loads(data)
    ctr = 0
    for fn in d.get('functions', []):
        for bb in fn.get('blocks', []):
            out = []
            for ins in bb.get('instructions', []):
                si = ins.get('sync_info')
                waits = (si or {}).get('on_wait') or []
                keep = _keep_count(ins)
                if len(waits) > keep:
                    hoist = waits[:len(waits) - keep]
                    keep_w = waits[len(waits) - keep:]
                    for w in hoist:
                        ctr += 1
                        nop = {
                            'name': f"{ins['name']}-ws{ctr}",
                            'opcode': 'NoOp',
                            'engine': ins.get('engine'),
                            'ins': [],
                            'outs': [],
                            'sync_info': {'on_wait': [w], 'on_update': []},
                        }
                        if 'debug' in ins:
                            nop['debug'] = ins['debug']
                        out.append(nop)
                    si['on_wait'] = keep_w
                out.append(ins)
            bb['instructions'] = out
    return orjson.dumps(d)


def _install_waitsplit():
    if getattr(bass.Bass, '_waitsplit_installed', False):
        return
    orig = bass.Bass.to_json_bytes

    def patched(self, *a, **k):
        return _split_waits_json(orig(self, *a, **k))

    bass.Bass.to_json_bytes = patched
    bass.Bass._waitsplit_installed = True


_install_waitsplit()


# ---------------------------------------------------------------------------
# Device program (SPMD, identical on all cores; per-core data differs)
# ---------------------------------------------------------------------------

def build_nc(s=S, d=D, hpc=HPC):
    lf = hpc * HD
    kd_n = d // 128          # contraction chunks for projections
    nw = 512 if s >= 512 else s  # free-dim width per matmul
    nsq = s // nw            # wide column chunks
    ns = s // 128            # 128-row chunks (key chunks)
    nj = d // 512 if d >= 512 else 1
    jw = 512 if d >= 512 else d
    scale = 1.0 / math.sqrt(HD)
    gw = min(4, ns)          # key chunks per batched exp ACTIVATE
    ngrp = ns // gw
    nunits = nsq * hpc
    nsub = nw // 128

    # All DRAM tensors are host-packed into SBUF layout ([128, ...] with the
    # kd/row blocks along the free dim) so every DMA moves >=4KB contiguous
    # per partition — 1KB-segment DMAs are descriptor-dominated (~30% of
    # peak) and were the startup bottleneck. y is packed the same way and
    # unpacked on the host.
    nc = bass.Bass()
    xT = nc.dram_tensor("xT", [128, kd_n * s], BF16, kind="ExternalInput")
    wqT = nc.dram_tensor("wqT", [128, kd_n * lf], BF16, kind="ExternalInput")
    wkT = nc.dram_tensor("wkT", [128, kd_n * lf], BF16, kind="ExternalInput")
    wvT = nc.dram_tensor("wvT", [128, kd_n * lf], BF16, kind="ExternalInput")
    woT = nc.dram_tensor("woT", [128, hpc * d], BF16, kind="ExternalInput")
    csd = nc.dram_tensor("csd", [128, s], BF16, kind="ExternalInput")
    snd = nc.dram_tensor("snd", [128, s], BF16, kind="ExternalInput")
    y = nc.dram_tensor("y", [128, (s // 128) * d], BF16, kind="ExternalOutput")
    # weight loads in blocks of up to 4 kd-chunks: big enough for efficient
    # DMA, small enough that the first matmuls start early
    wblk = min(4, kd_n)
    nwblk = kd_n // wblk

    with TileContext(nc) as tc:
        # Persistent SBUF residents: post-RoPE q/k (head-major), v (s-chunk
        # blocks), and the warm-up operand.
        with tc.tile_pool(name="persist", bufs=1) as per:
            qT_all = per.tile([128, hpc * s], BF16, name="qT_all")
            kT_all = per.tile([128, hpc * s], BF16, name="kT_all")
            v_all = per.tile([128, ns * lf], BF16, name="v_all")
            ones_b = per.tile([128, 128], BF16, name="ones_b")
            nc.vector.memset(ones_b, 1.0)

            # ---------- Stage A: q/k/v projections + RoPE (x streamed once) ----------
            with tc.tile_pool(name="wqk", bufs=1) as wpool, \
                 tc.tile_pool(name="csp", bufs=1) as cspool, \
                 tc.tile_pool(name="rp", bufs=2) as rpool, \
                 tc.tile_pool(name="psQ", bufs=5, space="PSUM") as psq_pool, \
                 tc.tile_pool(name="psW", bufs=1, space="PSUM") as pswarm, \
                 tc.tile_pool(name="psV", bufs=2, space="PSUM") as psv_pool:
                # PE clock warm-up: the startup is HBM-bound, so dummy
                # matmuls keep the HAM activity window non-idle (clock at
                # full rate) until the real matmuls flow. An initial batch
                # covers engine init; more are sprinkled through the first
                # chunk's DMA-paced phase below.
                wps = pswarm.tile([128, 128], F32, name="wps")
                for _ in range(48):
                    nc.tensor.matmul(wps, ones_b, ones_b, start=True, stop=True)
                wq_sb = wpool.tile([128, kd_n * lf], BF16, name="wq_sb")
                wk_sb = wpool.tile([128, kd_n * lf], BF16, name="wk_sb")
                wv_sb = wpool.tile([128, kd_n * lf], BF16, name="wv_sb")
                x_all = wpool.tile([128, kd_n * s], BF16, name="x_all")

                # cos/sin ride the gpsimd (SWDGE) queue so they don't delay
                # the wq/x stream on the sync queue
                cs_sb = cspool.tile([128, s], BF16, name="cs_sb")
                sn_sb = cspool.tile([128, s], BF16, name="sn_sb")
                nc.gpsimd.dma_start(out=cs_sb, in_=csd[:, :])
                nc.gpsimd.dma_start(out=sn_sb, in_=snd[:, :])
                # load order = consumption order. x is packed sq-outermost,
                # so each query chunk is one fat contiguous DMA; chunk 0 is
                # split in half so the first matmuls start a bit earlier.
                # sync queue: wq blocks + x chunks (the critical path);
                # scalar queue: cos/sin then wk; wv deferred past chunk 0.
                xw = kd_n * nw  # columns of one packed x chunk
                nc.sync.dma_start(out=wq_sb[:, 0:wblk * lf],
                                  in_=wqT[:, 0:wblk * lf])
                nc.sync.dma_start(out=x_all[:, 0:xw // 2],
                                  in_=xT[:, 0:xw // 2])
                if kd_n > wblk:
                    nc.sync.dma_start(out=wq_sb[:, wblk * lf:2 * wblk * lf],
                                      in_=wqT[:, wblk * lf:2 * wblk * lf])
                nc.sync.dma_start(out=x_all[:, xw // 2:xw],
                                  in_=xT[:, xw // 2:xw])
                for b0 in range(2 * wblk, kd_n, wblk):
                    nc.sync.dma_start(
                        out=wq_sb[:, b0 * lf:(b0 + wblk) * lf],
                        in_=wqT[:, b0 * lf:(b0 + wblk) * lf])
                for sq in range(1, nsq):
                    nc.sync.dma_start(out=x_all[:, sq * xw:(sq + 1) * xw],
                                      in_=xT[:, sq * xw:(sq + 1) * xw])
                for b0 in range(0, kd_n, wblk):
                    nc.scalar.dma_start(
                        out=wk_sb[:, b0 * lf:(b0 + wblk) * lf],
                        in_=wkT[:, b0 * lf:(b0 + wblk) * lf])

                def emit_v(sq):
                    # v for chunk sq, pipelined one chunk behind q/k: wv is the
                    # last weight to arrive and v isn't needed until stage B
                    for ss in range(nw // 128):
                        psv = psv_pool.tile([128, lf], F32, name="psv")
                        for kd in range(kd_n):
                            nc.tensor.matmul(
                                psv,
                                x_all[:, (sq * kd_n + kd) * nw + ss * 128:
                                      (sq * kd_n + kd) * nw + (ss + 1) * 128],
                                wv_sb[:, kd * lf:(kd + 1) * lf],
                                start=(kd == 0), stop=(kd == kd_n - 1))
                        nc.vector.tensor_copy(
                            v_all[:, (sq * (nw // 128) + ss) * lf:
                                  (sq * (nw // 128) + ss + 1) * lf], psv)

                for sq in range(nsq):
                    # k before q on the last chunk so stage B's first scores
                    # (which need ALL of k but only chunk 0 of q) start sooner
                    phases = ((wq_sb, qT_all), (wk_sb, kT_all))
                    if sq == nsq - 1:
                        phases = (phases[1], phases[0])
                    for wsb, dstT in phases:
                        # kd-major accumulation into per-head PSUM tiles: the
                        # first chunk's matmuls start as soon as each kd block
                        # of the weights/x arrives instead of after the whole
                        # tile
                        ps_h = [psq_pool.tile([128, nw], F32, name="ps_qk")
                                for _ in range(hpc)]
                        for kd in range(kd_n):
                            if sq == 0 and kd % wblk == 0:
                                # chunk 0 is DMA-paced: dummy matmuls between
                                # the weight-block boundaries keep the PE
                                # clock warm through the wait slivers
                                for _ in range(2):
                                    nc.tensor.matmul(wps, ones_b, ones_b,
                                                     start=True, stop=True)
                            for h in range(hpc):
                                nc.tensor.matmul(
                                    ps_h[h],
                                    wsb[:, kd * lf + h * 128: kd * lf + (h + 1) * 128],
                                    x_all[:, (sq * kd_n + kd) * nw:
                                          (sq * kd_n + kd + 1) * nw],
                                    start=(kd == 0), stop=(kd == kd_n - 1))
                        for h in range(hpc):
                            ps = ps_h[h]
                            tcc = rpool.tile([128, nw], F32, name="t_c")
                            tss = rpool.tile([128, nw], F32, name="t_s")
                            nc.vector.tensor_mul(tcc, ps, cs_sb[:, sq * nw:(sq + 1) * nw])
                            # sn_sb rows are [+sin; -sin]: after the half-swap the
                            # signed cross terms land with the right signs
                            nc.vector.tensor_mul(tss, ps, sn_sb[:, sq * nw:(sq + 1) * nw])
                            tsw = rpool.tile([128, nw], F32, name="t_sw")
                            nc.gpsimd.dma_start(out=tsw[0:64, :], in_=tss[64:128, :])
                            nc.gpsimd.dma_start(out=tsw[64:128, :], in_=tss[0:64, :])
                            nc.vector.tensor_add(
                                dstT[:, h * s + sq * nw: h * s + sq * nw + nw], tcc, tsw)
                    if sq == min(1, nsq - 1):
                        # wv after chunk 1: first needed by emit_v(0) at the
                        # end of chunk 1, far past the startup burst
                        for b0 in range(0, kd_n, wblk):
                            nc.scalar.dma_start(
                                out=wv_sb[:, b0 * lf:(b0 + wblk) * lf],
                                in_=wvT[:, b0 * lf:(b0 + wblk) * lf])
                    if sq > 0:
                        emit_v(sq - 1)
                emit_v(nsq - 1)

            # keep the PE busy (clock warm) across the stage boundary while
            # the last k-chunk RoPE drains on the DVE
            with tc.tile_pool(name="psW2", bufs=1, space="PSUM") as pswarm2:
                wps2 = pswarm2.tile([128, 128], F32, name="wps2")
                for _ in range(40):
                    nc.tensor.matmul(wps2, ones_b, ones_b, start=True, stop=True)

            # ---------- Stage B+C: attention, then out-proj per query chunk ----------
            with tc.tile_pool(name="exp", bufs=2) as expool, \
                 tc.tile_pool(name="fld", bufs=2) as fpool, \
                 tc.tile_pool(name="nrm", bufs=2) as npool, \
                 tc.tile_pool(name="atp", bufs=2) as atpool, \
                 tc.tile_pool(name="wop", bufs=1) as wopool, \
                 tc.tile_pool(name="yop", bufs=3) as yopool, \
                 tc.tile_pool(name="psS", bufs=1, space="PSUM") as pssc, \
                 tc.tile_pool(name="psO", bufs=1, space="PSUM") as psov, \
                 tc.tile_pool(name="psM", bufs=1, space="PSUM") as pssm, \
                 tc.tile_pool(name="psC", bufs=2, space="PSUM") as psc:
                wo_sb = wopool.tile([128, hpc * d], BF16, name="wo_sb")
                for i in range(hpc):
                    nc.scalar.dma_start(out=wo_sb[:, i * d:(i + 1) * d],
                                        in_=woT[:, i * d:(i + 1) * d])

                def scores_exp_group(u, ex_tile, g):
                    # one [128, gw*nw] PSUM group of scores for unit u and its
                    # batched exp ACTIVATE (amortizes the ACT fixed cost)
                    sq, h = divmod(u, hpc)
                    qT_sl = qT_all[:, h * s + sq * nw: h * s + (sq + 1) * nw]
                    sps = pssc.tile([128, gw * nw], F32, name="sps")
                    for j in range(gw):
                        sk = gw * g + j
                        nc.tensor.matmul(
                            sps[:, j * nw:(j + 1) * nw],
                            kT_all[:, h * s + sk * 128: h * s + (sk + 1) * 128],
                            qT_sl, start=True, stop=True)
                    nc.scalar.activation(
                        ex_tile[:, g * gw * nw:(g + 1) * gw * nw], sps,
                        mybir.ActivationFunctionType.Exp, scale=scale)

                def outproj_ops(psq, aT_tile, ssubs):
                    # out-projection micro-ops (one matmul or one cast each)
                    # for the given query-row slices of chunk psq; the caller
                    # interleaves them into PE wait slivers. The jn slices of
                    # one row-slice cast into a single [128, d] row tile which
                    # is stored with one wide (DMA-efficient) transfer.
                    ops = []
                    for ssub in ssubs:
                        yo_row = yopool.tile([128, d], BF16, name="yo_row")
                        for jn in range(nj):
                            yps = psc.tile([128, jw], F32, name="yps")
                            for i in range(hpc):
                                ops.append(
                                    lambda yps=yps, i=i, jn=jn, ssub=ssub:
                                    nc.tensor.matmul(
                                        yps,
                                        aT_tile[:, i * nw + ssub * 128:
                                                i * nw + (ssub + 1) * 128],
                                        wo_sb[:, i * d + jn * jw:
                                              i * d + (jn + 1) * jw],
                                        start=(i == 0), stop=(i == hpc - 1)))

                            def fin(yps=yps, jn=jn, ssub=ssub, yo_row=yo_row):
                                # split the PSUM->SBUF bf16 casts between ACT
                                # and DVE so neither becomes the bottleneck
                                if jn % 2 == 0:
                                    nc.scalar.copy(
                                        yo_row[:, jn * jw:(jn + 1) * jw], yps)
                                else:
                                    nc.vector.tensor_copy(
                                        yo_row[:, jn * jw:(jn + 1) * jw], yps)
                                if jn == nj - 1:
                                    row = psq * nsub + ssub
                                    nc.sync.dma_start(
                                        out=y[:, row * d:(row + 1) * d],
                                        in_=yo_row)
                            ops.append(fin)
                    return ops

                def fold_push(stack, ap, lvl):
                    # binary-counter combine: same depth-log2 rounding as a
                    # balanced tree, but each combine runs as soon as its two
                    # inputs exist — the last one lands right after the last
                    # exp instead of a full tree-latency later
                    while stack and stack[-1][1] == lvl:
                        prev, _ = stack.pop()
                        t = fpool.tile([128, gw * nw], BF16, name=f"fold{lvl}")
                        nc.vector.tensor_add(t, prev, ap)
                        ap, lvl = t, lvl + 1
                    stack.append((ap, lvl))

                def fold_finish(stack):
                    ap, _ = stack.pop()
                    while stack:
                        prev, pl = stack.pop()
                        t = fpool.tile([128, gw * nw], BF16, name=f"fold{pl}")
                        nc.vector.tensor_add(t, prev, ap)
                        ap = t
                    width = gw * nw
                    while width > 2 * nw:
                        t = fpool.tile([128, width // 2], BF16,
                                       name=f"half{width}")
                        nc.vector.tensor_add(t, ap[:, :width // 2],
                                             ap[:, width // 2:width])
                        ap, width = t, width // 2
                    accb = npool.tile([128, nw], BF16, name="accb")
                    nc.vector.tensor_add(accb, ap[:, :nw], ap[:, nw:2 * nw])
                    return accb

                def emit_folds(ex_u):
                    # post-hoc variant (used only for unit 0's tile, emitted
                    # before the main loop)
                    stack = []
                    for g in range(ngrp):
                        fold_push(stack,
                                  ex_u[:, g * gw * nw:(g + 1) * gw * nw], 0)
                    return fold_finish(stack)

                ex_tiles = {}
                ex_tiles[0] = expool.tile([128, ns * nw], BF16, name="ex_sb")
                for g in range(ngrp):
                    scores_exp_group(0, ex_tiles[0], g)
                accb_tiles = {0: emit_folds(ex_tiles[0])}
                prev_c = None  # (sq, aT_tile) of the previous chunk
                aT_sq = None
                for u in range(nunits):
                    sq, h = divmod(u, hpc)
                    if h == 0:
                        aT_sq = atpool.tile([128, hpc * nw], BF16, name="aT_sq")
                    # denominator part 2 first: by block start the fold tree
                    # for this unit is done (it ran during the previous
                    # block), so the ones-matmul (partition reduction +
                    # broadcast) and the reciprocal clear immediately and
                    # nothing downstream waits on the DVE late in the block.
                    # Exception: at u=0 the fold tree only starts with stage
                    # B, so it would head the PE queue and block everything.
                    def emit_sm_recip():
                        sm = pssm.tile([128, nw], F32, name="sm")
                        nc.tensor.matmul(sm, ones_b, accb_tiles.pop(u),
                                         start=True, stop=True)
                        rec = npool.tile([128, nw], F32, name="rec")
                        nc.vector.reciprocal(rec, sm)
                        return rec

                    if u > 0:
                        rec = emit_sm_recip()
                    # out-projection micro-ops of the PREVIOUS chunk for this
                    # unit's row slices, to be interleaved below
                    if prev_c is not None:
                        psq, pat = prev_c
                        ops = outproj_ops(
                            psq, pat,
                            range(h * nsub // hpc, (h + 1) * nsub // hpc))
                    else:
                        ops = []
                    # interleave per score-pair: the next unit's scores+exp
                    # (paced by the ACT chain via the PSUM ring), this unit's
                    # PV matmuls, and the out-projection micro-ops fill the
                    # PE slivers in between
                    if u + 1 < nunits:
                        ex_tiles[u + 1] = expool.tile([128, ns * nw], BF16,
                                                      name="ex_sb")
                    ex_u = ex_tiles.pop(u)
                    ov = psov.tile([128, nw], F32, name="ov")
                    oi = 0
                    fold_stack = []
                    for g in range(ngrp):
                        if u + 1 < nunits:
                            scores_exp_group(u + 1, ex_tiles[u + 1], g)
                            # progressive fold of the prefetched exp groups:
                            # the denominator for unit u+1 is complete right
                            # after its last exp, so the next block's
                            # ones-matmul never stalls the PE queue
                            fold_push(
                                fold_stack,
                                ex_tiles[u + 1][:, g * gw * nw:
                                                (g + 1) * gw * nw],
                                0)
                        for j in range(gw):
                            sk = gw * g + j
                            nc.tensor.matmul(ov,
                                             v_all[:, sk * lf + h * 128:
                                                   sk * lf + (h + 1) * 128],
                                             ex_u[:, sk * nw:(sk + 1) * nw],
                                             start=(sk == 0),
                                             stop=(sk == ns - 1))
                        take = (((g + 1) * len(ops)) // ngrp
                                - (g * len(ops)) // ngrp)
                        for _ in range(take):
                            ops[oi]()
                            oi += 1
                        if not ops:
                            # first chunk has no out-projection yet: standalone
                            # weight loads keep the PE activity window non-idle
                            # (clock warm) through the ACT-paced wait slivers
                            for _ in range(8):
                                nc.tensor.ldweights(ones_b)
                    if u == 0:
                        rec = emit_sm_recip()
                    nc.vector.tensor_mul(aT_sq[:, h * nw:(h + 1) * nw], ov, rec)
                    if u + 1 < nunits:
                        accb_tiles[u + 1] = fold_finish(fold_stack)
                    if h == hpc - 1:
                        prev_c = (sq, aT_sq)
                # drain the final chunk's out-projection
                psq, pat = prev_c
                for op in outproj_ops(psq, pat, range(nsub)):
                    op()
    return nc


# ---------------------------------------------------------------------------
# Host-side sharding + gather
# ---------------------------------------------------------------------------

_PERM_HEAD = np.concatenate([np.arange(0, HD, 2), np.arange(1, HD, 2)])


def _pack_rows(a):
    """[n*128, m] -> [128, n*m]: kd-blocks of 128 rows side by side along the
    free dim — the SBUF-resident layout, so device DMAs are contiguous."""
    n = a.shape[0] // 128
    return np.ascontiguousarray(
        a.reshape(n, 128, a.shape[1]).transpose(1, 0, 2).reshape(128, -1))


def _unpack_y(yp, s, d):
    """[128, (s//128)*d] -> [s, d] (inverse of the device's packed store)."""
    n = s // 128
    return yp.reshape(128, n, d).transpose(1, 0, 2).reshape(s, d)


def _prep_in_maps(x, wq, wk, wv, wo, pos_cos, pos_sin, s=S, d=D, hpc=HPC):
    lf = hpc * HD
    h_total = d // HD
    groups = h_total // hpc
    # permute q/k feature rows within each head: even pairs first, then odd
    wq_p = wq.reshape(h_total, HD, d)[:, _PERM_HEAD, :].reshape(d, d)
    wk_p = wk.reshape(h_total, HD, d)[:, _PERM_HEAD, :].reshape(d, d)
    wqT_full = np.ascontiguousarray(wq_p.T).astype(NP_BF16)
    wkT_full = np.ascontiguousarray(wk_p.T).astype(NP_BF16)
    wvT_full = np.ascontiguousarray(wv.T).astype(NP_BF16)
    woT_full = np.ascontiguousarray(wo.T).astype(NP_BF16)
    cs_half = np.ascontiguousarray(pos_cos[0].T).astype(np.float32)  # [64, S]
    sn_half = np.ascontiguousarray(pos_sin[0].T).astype(np.float32)
    csd = np.concatenate([cs_half, cs_half], axis=0).astype(NP_BF16)
    snd = np.concatenate([sn_half, -sn_half], axis=0).astype(NP_BF16)
    in_maps = []
    n_batches = x.shape[0]
    # x packed sq-outermost: [128, sq][kd][nw] so each query chunk of every
    # contraction block is one contiguous device DMA
    kd_n = d // 128
    nw = 512 if s >= 512 else s
    nsq = s // nw

    def pack_x(xb):
        xt = np.ascontiguousarray(xb.T).astype(NP_BF16)  # [d, s]
        return np.ascontiguousarray(
            xt.reshape(kd_n, 128, nsq, nw).transpose(1, 2, 0, 3)
            .reshape(128, kd_n * s))

    xP = [pack_x(x[b]) for b in range(n_batches)]
    for c in range(n_batches * groups):
        b, g = divmod(c, groups)
        in_maps.append({
            "xT": xP[b],
            "wqT": _pack_rows(wqT_full[:, g * lf:(g + 1) * lf]),
            "wkT": _pack_rows(wkT_full[:, g * lf:(g + 1) * lf]),
            "wvT": _pack_rows(wvT_full[:, g * lf:(g + 1) * lf]),
            "woT": _pack_rows(woT_full[g * lf:(g + 1) * lf, :]),
            "csd": csd,
            "snd": snd,
        })
    return in_maps


_NC_CACHE = {}


def _get_nc(s=S, d=D, hpc=HPC):
    key = (s, d, hpc)
    if key not in _NC_CACHE:
        _NC_CACHE[key] = build_nc(s, d, hpc)
    return _NC_CACHE[key]


def _np_rope(t, cos, sin):
    b, ss, hh, hd = t.shape
    tr = t.reshape(b, ss, hh, hd // 2, 2)
    te, to = tr[..., 0], tr[..., 1]
    c = cos[:, :, None, :]
    s = sin[:, :, None, :]
    return np.stack([te * c - to * s, te * s + to * c], axis=-1).reshape(b, ss, hh, hd)


def _score_sample_max(x, wq, wk, pos_cos, pos_sin):
    """Sampled estimate of max |score|; the device softmax skips the max
    subtraction, which is only safe when scores stay well under exp's fp32
    range."""
    ss = x[:, :: max(1, x.shape[1] // 32), :][:, :32]
    pos_idx = np.arange(x.shape[1])[:: max(1, x.shape[1] // 32)][:32]
    h = x.shape[2] // HD
    q = (ss @ wq.T).reshape(ss.shape[0], -1, h, HD)
    k = (ss @ wk.T).reshape(ss.shape[0], -1, h, HD)
    c = pos_cos[:, pos_idx]
    sn = pos_sin[:, pos_idx]
    q = _np_rope(q, c, sn)
    k = _np_rope(k, c, sn)
    sc = np.einsum('bqhd,bkhd->bhqk', q, k) / math.sqrt(HD)
    return float(np.abs(sc).max())


def _np_fallback(x, wq, wk, wv, wo, pos_cos, pos_sin):
    out = np.empty_like(x)
    h = x.shape[2] // HD
    for b in range(x.shape[0]):
        q = _np_rope((x[b:b + 1] @ wq.T).reshape(1, -1, h, HD), pos_cos, pos_sin)
        k = _np_rope((x[b:b + 1] @ wk.T).reshape(1, -1, h, HD), pos_cos, pos_sin)
        v = (x[b:b + 1] @ wv.T).reshape(1, -1, h, HD)
        sc = np.einsum('bqhd,bkhd->bhqk', q, k) / math.sqrt(HD)
        sc -= sc.max(axis=-1, keepdims=True)
        e = np.exp(sc, dtype=np.float32)
        p = e / e.sum(axis=-1, keepdims=True)
        out[b] = (np.einsum('bhqk,bkhd->bqhd', p, v).reshape(1, x.shape[1], -1)
                  @ wo.T)[0]
    return out


def kernel(x, wq, wk, wv, wo, pos_cos, pos_sin):
    x = np.asarray(x, dtype=np.float32)
    wq, wk, wv, wo = (np.asarray(a, dtype=np.float32) for a in (wq, wk, wv, wo))
    pos_cos = np.asarray(pos_cos, dtype=np.float32)
    pos_sin = np.asarray(pos_sin, dtype=np.float32)
    # the device softmax skips max subtraction (safe for scores ~ N(0,1));
    # if the inputs are scaled such that exp would overflow, fall back to a
    # correct (slower) host path rather than returning inf/NaN
    if 4.0 * _score_sample_max(x, wq, wk, pos_cos, pos_sin) > 80.0:
        return _np_fallback(x, wq, wk, wv, wo, pos_cos, pos_sin)
    in_maps = _prep_in_maps(x, wq, wk, wv, wo, pos_cos, pos_sin)
    nc = _get_nc()
    res = run_bass_kernel_spmd(nc, in_maps, core_ids=list(range(N_CORES)))
    out = np.empty((B, S, D), dtype=np.float32)
    for b in range(B):
        acc = _unpack_y(res.results[b * GROUPS]["y"].astype(np.float32), S, D)
        for g in range(1, GROUPS):
            acc = acc + _unpack_y(
                res.results[b * GROUPS + g]["y"].astype(np.float32), S, D)
        out[b] = acc
    return out


# revision 54
# speedup vs baseline: 1.2956x; 1.1815x over previous
"""Multi-head attention (RoPE, softmax, out-proj) on 8 Trainium2 NeuronCores.

Sharding: batch (2) x head-groups (4) -> 8 cores. Each core computes, for its
batch b and its 4 heads: q/k/v projections (column-parallel), RoPE, full
attention, and a partial output projection against its slice of wo
(row-parallel). The 4 partial outputs per batch are summed on the host.

Matmuls run in bf16 (full PE rate, FWL weight loads) with fp32 PSUM
accumulation. The softmax is computed unnormalized (exp without max
subtraction is safe: scores ~ N(0,1)); the denominator is a bf16 halving
tree on the DVE over the exp tiles followed by a fast-approx reciprocal.

Layout trick: weights are pre-transposed on the host so every matmul operand
is a natural [contraction-dim-major] DMA. Within each head, q/k feature rows
are permuted to (even pairs, odd pairs) so RoPE's interleaved pair structure
becomes a partition-block structure (rows 0:64 / 64:128); scores are
invariant to the (shared) permutation and v/wo stay unpermuted. The halves
swap needed by RoPE's cross terms is done with two SBUF->SBUF DMAs on the
(otherwise idle) gpsimd queue and the signs are folded into the
(host-prepared) sin rows [+sin; -sin].

Stage B is software-pipelined per (query-chunk, head) unit: the PE issues the
NEXT unit's score matmuls before the current unit's PV matmuls, so the ACT
exp chain never starves; exp runs on [128,1024] PSUM pairs to amortize the
per-ACTIVATE fixed cost. The out-projection for the previous query chunk is
interleaved one row-slice per unit; its PSUM->SBUF bf16 cast runs on gpsimd
and the store DMA on the sync queue.
"""
import math
import sys

import numpy as np

for _p in ('/opt/trn_rl_repo', '/root/.axon_site/_ro/trn_rl_repo'):
    if _p not in sys.path:
        sys.path.insert(0, _p)

import ml_dtypes
import orjson

import concourse.bass as bass
import concourse.mybir as mybir
from concourse.tile import TileContext
from concourse.bass_utils import run_bass_kernel_spmd

F32 = mybir.dt.float32
BF16 = mybir.dt.bfloat16
NP_BF16 = ml_dtypes.bfloat16

B = 2
S = 2048
D = 2048
HD = 128
N_CORES = 8
GROUPS = 4          # head groups (tensor-parallel degree per batch)
HPC = (D // HD) // GROUPS  # heads per core (4)
LF = HPC * HD       # local features per core (512)


# ---------------------------------------------------------------------------
# Wait-splitting post-pass: this toolchain's walrus supports at most ONE sync
# wait command per instruction (none at all on fp32/fp32r Matmult, which
# lowers to an LDW+MM pair). Tile emits multi-wait instructions; hoist the
# excess onto NoOps on the same engine immediately before the instruction.
# ---------------------------------------------------------------------------

def _keep_count(ins):
    if ins.get('opcode') == 'Matmult':
        dt = None
        for arg in ins.get('ins', []):
            dt = arg.get('dtype') or dt
        if dt in ('float32', 'float32r'):
            return 0
        return 1
    if ins.get('opcode') == 'ISA':
        # custom-DVE ISA instructions have a fixed encoding with no room
        # for a sync wait command
        return 0
    return 1


def _split_waits_json(data: bytes) -> bytes:
    d = orjson.loads(data)
    ctr = 0
    for fn in d.get('functions', []):
        for bb in fn.get('blocks', []):
            out = []
            for ins in bb.get('instructions', []):
                si = ins.get('sync_info')
                waits = (si or {}).get('on_wait') or []
                keep = _keep_count(ins)
                if len(waits) > keep:
                    hoist = waits[:len(waits) - keep]
                    keep_w = waits[len(waits) - keep:]
                    for w in hoist:
                        ctr += 1
                        nop = {
                            'name': f"{ins['name']}-ws{ctr}",
                            'opcode': 'NoOp',
                            'engine': ins.get('engine'),
                            'ins': [],
                            'outs': [],
                            'sync_info': {'on_wait': [w], 'on_update': []},
                        }
                        if 'debug' in ins:
                            nop['debug'] = ins['debug']
                        out.append(nop)
                    si['on_wait'] = keep_w
                out.append(ins)
            bb['instructions'] = out
    return orjson.dumps(d)


def _install_waitsplit():
    if getattr(bass.Bass, '_waitsplit_installed', False):
        return
    orig = bass.Bass.to_json_bytes

    def patched(self, *a, **k):
        return _split_waits_json(orig(self, *a, **k))

    bass.Bass.to_json_bytes = patched
    bass.Bass._waitsplit_installed = True


_install_waitsplit()


# ---------------------------------------------------------------------------
# Device program (SPMD, identical on all cores; per-core data differs)
# ---------------------------------------------------------------------------

def build_nc(s=S, d=D, hpc=HPC):
    lf = hpc * HD
    kd_n = d // 128          # contraction chunks for projections
    nw = 512 if s >= 512 else s  # free-dim width per matmul
    nsq = s // nw            # wide column chunks
    ns = s // 128            # 128-row chunks (key chunks)
    nj = d // 512 if d >= 512 else 1
    jw = 512 if d >= 512 else d
    scale = 1.0 / math.sqrt(HD)
    gw = 2                   # key chunks per batched exp ACTIVATE
    ngrp = ns // gw
    nunits = nsq * hpc
    nsub = nw // 128

    # All DRAM tensors are host-packed into SBUF layout ([128, ...] with the
    # kd/row blocks along the free dim) so every DMA moves >=4KB contiguous
    # per partition — 1KB-segment DMAs are descriptor-dominated (~30% of
    # peak) and were the startup bottleneck. y is packed the same way and
    # unpacked on the host.
    nc = bass.Bass()
    xT = nc.dram_tensor("xT", [128, kd_n * s], BF16, kind="ExternalInput")
    wqT = nc.dram_tensor("wqT", [128, kd_n * lf], BF16, kind="ExternalInput")
    wkT = nc.dram_tensor("wkT", [128, kd_n * lf], BF16, kind="ExternalInput")
    wvT = nc.dram_tensor("wvT", [128, kd_n * lf], BF16, kind="ExternalInput")
    woT = nc.dram_tensor("woT", [128, hpc * d], BF16, kind="ExternalInput")
    csd = nc.dram_tensor("csd", [128, s], BF16, kind="ExternalInput")
    snd = nc.dram_tensor("snd", [128, s], BF16, kind="ExternalInput")
    y = nc.dram_tensor("y", [128, (s // 128) * d], BF16, kind="ExternalOutput")
    # weight loads in blocks of up to 4 kd-chunks: big enough for efficient
    # DMA, small enough that the first matmuls start early
    wblk = min(4, kd_n)
    nwblk = kd_n // wblk

    with TileContext(nc) as tc:
        # Persistent SBUF residents: post-RoPE q/k (head-major), v (s-chunk
        # blocks), and the warm-up operand.
        with tc.tile_pool(name="persist", bufs=1) as per:
            qT_all = per.tile([128, hpc * s], BF16, name="qT_all")
            kT_all = per.tile([128, hpc * s], BF16, name="kT_all")
            v_all = per.tile([128, ns * lf], BF16, name="v_all")
            ones_b = per.tile([128, 128], BF16, name="ones_b")
            nc.vector.memset(ones_b, 1.0)

            # ---------- Stage A: q/k/v projections + RoPE (x streamed once) ----------
            with tc.tile_pool(name="wqk", bufs=1) as wpool, \
                 tc.tile_pool(name="csp", bufs=1) as cspool, \
                 tc.tile_pool(name="rp", bufs=2) as rpool, \
                 tc.tile_pool(name="psQ", bufs=5, space="PSUM") as psq_pool, \
                 tc.tile_pool(name="psW", bufs=1, space="PSUM") as pswarm, \
                 tc.tile_pool(name="psV", bufs=2, space="PSUM") as psv_pool:
                # PE clock warm-up: the startup is HBM-bound, so dummy
                # matmuls keep the HAM activity window non-idle (clock at
                # full rate) until the real matmuls flow. An initial batch
                # covers engine init; more are sprinkled through the first
                # chunk's DMA-paced phase below.
                wps = pswarm.tile([128, 128], F32, name="wps")
                for _ in range(48):
                    nc.tensor.matmul(wps, ones_b, ones_b, start=True, stop=True)
                wq_sb = wpool.tile([128, kd_n * lf], BF16, name="wq_sb")
                wk_sb = wpool.tile([128, kd_n * lf], BF16, name="wk_sb")
                wv_sb = wpool.tile([128, kd_n * lf], BF16, name="wv_sb")
                x_all = wpool.tile([128, kd_n * s], BF16, name="x_all")

                # cos/sin ride the gpsimd (SWDGE) queue so they don't delay
                # the wq/x stream on the sync queue
                cs_sb = cspool.tile([128, s], BF16, name="cs_sb")
                sn_sb = cspool.tile([128, s], BF16, name="sn_sb")
                nc.gpsimd.dma_start(out=cs_sb, in_=csd[:, :])
                nc.gpsimd.dma_start(out=sn_sb, in_=snd[:, :])
                # load order = consumption order. x is packed sq-outermost,
                # so each query chunk is one fat contiguous DMA; chunk 0 is
                # split in half so the first matmuls start a bit earlier.
                # sync queue: wq blocks + x chunks (the critical path);
                # scalar queue: cos/sin then wk; wv deferred past chunk 0.
                xw = kd_n * nw  # columns of one packed x chunk
                nc.sync.dma_start(out=wq_sb[:, 0:wblk * lf],
                                  in_=wqT[:, 0:wblk * lf])
                nc.sync.dma_start(out=x_all[:, 0:xw // 2],
                                  in_=xT[:, 0:xw // 2])
                if kd_n > wblk:
                    nc.sync.dma_start(out=wq_sb[:, wblk * lf:2 * wblk * lf],
                                      in_=wqT[:, wblk * lf:2 * wblk * lf])
                nc.sync.dma_start(out=x_all[:, xw // 2:xw],
                                  in_=xT[:, xw // 2:xw])
                for b0 in range(2 * wblk, kd_n, wblk):
                    nc.sync.dma_start(
                        out=wq_sb[:, b0 * lf:(b0 + wblk) * lf],
                        in_=wqT[:, b0 * lf:(b0 + wblk) * lf])
                for sq in range(1, nsq):
                    nc.sync.dma_start(out=x_all[:, sq * xw:(sq + 1) * xw],
                                      in_=xT[:, sq * xw:(sq + 1) * xw])
                for b0 in range(0, kd_n, wblk):
                    nc.scalar.dma_start(
                        out=wk_sb[:, b0 * lf:(b0 + wblk) * lf],
                        in_=wkT[:, b0 * lf:(b0 + wblk) * lf])

                def emit_v(sq):
                    # v for chunk sq, pipelined one chunk behind q/k: wv is the
                    # last weight to arrive and v isn't needed until stage B
                    for ss in range(nw // 128):
                        psv = psv_pool.tile([128, lf], F32, name="psv")
                        for kd in range(kd_n):
                            nc.tensor.matmul(
                                psv,
                                x_all[:, (sq * kd_n + kd) * nw + ss * 128:
                                      (sq * kd_n + kd) * nw + (ss + 1) * 128],
                                wv_sb[:, kd * lf:(kd + 1) * lf],
                                start=(kd == 0), stop=(kd == kd_n - 1))
                        nc.vector.tensor_copy(
                            v_all[:, (sq * (nw // 128) + ss) * lf:
                                  (sq * (nw // 128) + ss + 1) * lf], psv)

                for sq in range(nsq):
                    # k before q on the last chunk so stage B's first scores
                    # (which need ALL of k but only chunk 0 of q) start sooner
                    phases = ((wq_sb, qT_all), (wk_sb, kT_all))
                    if sq == nsq - 1:
                        phases = (phases[1], phases[0])
                    for wsb, dstT in phases:
                        # kd-major accumulation into per-head PSUM tiles: the
                        # first chunk's matmuls start as soon as each kd block
                        # of the weights/x arrives instead of after the whole
                        # tile
                        ps_h = [psq_pool.tile([128, nw], F32, name="ps_qk")
                                for _ in range(hpc)]
                        for kd in range(kd_n):
                            if sq == 0 and kd % wblk == 0:
                                # chunk 0 is DMA-paced: dummy matmuls between
                                # the weight-block boundaries keep the PE
                                # clock warm through the wait slivers
                                for _ in range(2):
                                    nc.tensor.matmul(wps, ones_b, ones_b,
                                                     start=True, stop=True)
                            for h in range(hpc):
                                nc.tensor.matmul(
                                    ps_h[h],
                                    wsb[:, kd * lf + h * 128: kd * lf + (h + 1) * 128],
                                    x_all[:, (sq * kd_n + kd) * nw:
                                          (sq * kd_n + kd + 1) * nw],
                                    start=(kd == 0), stop=(kd == kd_n - 1))
                        for h in range(hpc):
                            ps = ps_h[h]
                            tcc = rpool.tile([128, nw], F32, name="t_c")
                            tss = rpool.tile([128, nw], F32, name="t_s")
                            nc.vector.tensor_mul(tcc, ps, cs_sb[:, sq * nw:(sq + 1) * nw])
                            # sn_sb rows are [+sin; -sin]: after the half-swap the
                            # signed cross terms land with the right signs
                            nc.vector.tensor_mul(tss, ps, sn_sb[:, sq * nw:(sq + 1) * nw])
                            tsw = rpool.tile([128, nw], F32, name="t_sw")
                            nc.gpsimd.dma_start(out=tsw[0:64, :], in_=tss[64:128, :])
                            nc.gpsimd.dma_start(out=tsw[64:128, :], in_=tss[0:64, :])
                            nc.vector.tensor_add(
                                dstT[:, h * s + sq * nw: h * s + sq * nw + nw], tcc, tsw)
                    if sq == min(1, nsq - 1):
                        # wv after chunk 1: first needed by emit_v(0) at the
                        # end of chunk 1, far past the startup burst
                        for b0 in range(0, kd_n, wblk):
                            nc.scalar.dma_start(
                                out=wv_sb[:, b0 * lf:(b0 + wblk) * lf],
                                in_=wvT[:, b0 * lf:(b0 + wblk) * lf])
                    if sq > 0:
                        emit_v(sq - 1)
                emit_v(nsq - 1)

            # keep the PE busy (clock warm) across the stage boundary while
            # the last k-chunk RoPE drains on the DVE
            with tc.tile_pool(name="psW2", bufs=1, space="PSUM") as pswarm2:
                wps2 = pswarm2.tile([128, 128], F32, name="wps2")
                for _ in range(40):
                    nc.tensor.matmul(wps2, ones_b, ones_b, start=True, stop=True)

            # ---------- Stage B+C: attention, then out-proj per query chunk ----------
            with tc.tile_pool(name="exp", bufs=2) as expool, \
                 tc.tile_pool(name="fld", bufs=2) as fpool, \
                 tc.tile_pool(name="nrm", bufs=2) as npool, \
                 tc.tile_pool(name="atp", bufs=2) as atpool, \
                 tc.tile_pool(name="wop", bufs=1) as wopool, \
                 tc.tile_pool(name="yop", bufs=3) as yopool, \
                 tc.tile_pool(name="psS", bufs=2, space="PSUM") as pssc, \
                 tc.tile_pool(name="psO", bufs=1, space="PSUM") as psov, \
                 tc.tile_pool(name="psM", bufs=1, space="PSUM") as pssm, \
                 tc.tile_pool(name="psC", bufs=2, space="PSUM") as psc:
                wo_sb = wopool.tile([128, hpc * d], BF16, name="wo_sb")
                for i in range(hpc):
                    nc.scalar.dma_start(out=wo_sb[:, i * d:(i + 1) * d],
                                        in_=woT[:, i * d:(i + 1) * d])

                def scores_exp_group(u, ex_tile, g):
                    # one [128, gw*nw] PSUM group of scores for unit u and its
                    # batched exp ACTIVATE (amortizes the ACT fixed cost)
                    sq, h = divmod(u, hpc)
                    qT_sl = qT_all[:, h * s + sq * nw: h * s + (sq + 1) * nw]
                    sps = pssc.tile([128, gw * nw], F32, name="sps")
                    for j in range(gw):
                        sk = gw * g + j
                        nc.tensor.matmul(
                            sps[:, j * nw:(j + 1) * nw],
                            kT_all[:, h * s + sk * 128: h * s + (sk + 1) * 128],
                            qT_sl, start=True, stop=True)
                    nc.scalar.activation(
                        ex_tile[:, g * gw * nw:(g + 1) * gw * nw], sps,
                        mybir.ActivationFunctionType.Exp, scale=scale)

                def outproj_ops(psq, aT_tile, ssubs):
                    # out-projection micro-ops (one matmul or one cast each)
                    # for the given query-row slices of chunk psq; the caller
                    # interleaves them into PE wait slivers. The jn slices of
                    # one row-slice cast into a single [128, d] row tile which
                    # is stored with one wide (DMA-efficient) transfer.
                    ops = []
                    for ssub in ssubs:
                        yo_row = yopool.tile([128, d], BF16, name="yo_row")
                        for jn in range(nj):
                            yps = psc.tile([128, jw], F32, name="yps")
                            for i in range(hpc):
                                ops.append(
                                    lambda yps=yps, i=i, jn=jn, ssub=ssub:
                                    nc.tensor.matmul(
                                        yps,
                                        aT_tile[:, i * nw + ssub * 128:
                                                i * nw + (ssub + 1) * 128],
                                        wo_sb[:, i * d + jn * jw:
                                              i * d + (jn + 1) * jw],
                                        start=(i == 0), stop=(i == hpc - 1)))

                            def fin(yps=yps, jn=jn, ssub=ssub, yo_row=yo_row):
                                # split the PSUM->SBUF bf16 casts between ACT
                                # and DVE so neither becomes the bottleneck
                                if jn % 2 == 0:
                                    nc.scalar.copy(
                                        yo_row[:, jn * jw:(jn + 1) * jw], yps)
                                else:
                                    nc.vector.tensor_copy(
                                        yo_row[:, jn * jw:(jn + 1) * jw], yps)
                                if jn == nj - 1:
                                    row = psq * nsub + ssub
                                    nc.sync.dma_start(
                                        out=y[:, row * d:(row + 1) * d],
                                        in_=yo_row)
                            ops.append(fin)
                    return ops

                def fold_push(stack, ap, lvl):
                    # binary-counter combine: same depth-log2 rounding as a
                    # balanced tree, but each combine runs as soon as its two
                    # inputs exist — the last one lands right after the last
                    # exp instead of a full tree-latency later
                    while stack and stack[-1][1] == lvl:
                        prev, _ = stack.pop()
                        t = fpool.tile([128, gw * nw], BF16, name=f"fold{lvl}")
                        nc.vector.tensor_add(t, prev, ap)
                        ap, lvl = t, lvl + 1
                    stack.append((ap, lvl))

                def fold_finish(stack):
                    ap, _ = stack.pop()
                    while stack:
                        prev, pl = stack.pop()
                        t = fpool.tile([128, gw * nw], BF16, name=f"fold{pl}")
                        nc.vector.tensor_add(t, prev, ap)
                        ap = t
                    width = gw * nw
                    while width > 2 * nw:
                        t = fpool.tile([128, width // 2], BF16,
                                       name=f"half{width}")
                        nc.vector.tensor_add(t, ap[:, :width // 2],
                                             ap[:, width // 2:width])
                        ap, width = t, width // 2
                    accb = npool.tile([128, nw], BF16, name="accb")
                    nc.vector.tensor_add(accb, ap[:, :nw], ap[:, nw:2 * nw])
                    return accb

                def emit_folds(ex_u):
                    # post-hoc variant (used only for unit 0's tile, emitted
                    # before the main loop)
                    stack = []
                    for g in range(ngrp):
                        fold_push(stack,
                                  ex_u[:, g * gw * nw:(g + 1) * gw * nw], 0)
                    return fold_finish(stack)

                ex_tiles = {}
                ex_tiles[0] = expool.tile([128, ns * nw], BF16, name="ex_sb")
                for g in range(ngrp):
                    scores_exp_group(0, ex_tiles[0], g)
                accb_tiles = {0: emit_folds(ex_tiles[0])}
                prev_c = None  # (sq, aT_tile) of the previous chunk
                aT_sq = None
                for u in range(nunits):
                    sq, h = divmod(u, hpc)
                    if h == 0:
                        aT_sq = atpool.tile([128, hpc * nw], BF16, name="aT_sq")
                    # denominator part 2 first: by block start the fold tree
                    # for this unit is done (it ran during the previous
                    # block), so the ones-matmul (partition reduction +
                    # broadcast) and the reciprocal clear immediately and
                    # nothing downstream waits on the DVE late in the block.
                    # Exception: at u=0 the fold tree only starts with stage
                    # B, so it would head the PE queue and block everything.
                    def emit_sm_recip():
                        sm = pssm.tile([128, nw], F32, name="sm")
                        nc.tensor.matmul(sm, ones_b, accb_tiles.pop(u),
                                         start=True, stop=True)
                        rec = npool.tile([128, nw], F32, name="rec")
                        nc.vector.reciprocal(rec, sm)
                        return rec

                    if u > 0:
                        rec = emit_sm_recip()
                    # out-projection micro-ops of the PREVIOUS chunk for this
                    # unit's row slices, to be interleaved below
                    if prev_c is not None:
                        psq, pat = prev_c
                        ops = outproj_ops(
                            psq, pat,
                            range(h * nsub // hpc, (h + 1) * nsub // hpc))
                    else:
                        ops = []
                    # interleave per score-pair: the next unit's scores+exp
                    # (paced by the ACT chain via the PSUM ring), this unit's
                    # PV matmuls, and the out-projection micro-ops fill the
                    # PE slivers in between
                    if u + 1 < nunits:
                        ex_tiles[u + 1] = expool.tile([128, ns * nw], BF16,
                                                      name="ex_sb")
                    ex_u = ex_tiles.pop(u)
                    ov = psov.tile([128, nw], F32, name="ov")
                    oi = 0
                    fold_stack = []
                    for g in range(ngrp):
                        if u + 1 < nunits:
                            scores_exp_group(u + 1, ex_tiles[u + 1], g)
                            # progressive fold of the prefetched exp groups:
                            # the denominator for unit u+1 is complete right
                            # after its last exp, so the next block's
                            # ones-matmul never stalls the PE queue
                            fold_push(
                                fold_stack,
                                ex_tiles[u + 1][:, g * gw * nw:
                                                (g + 1) * gw * nw],
                                0)
                        for j in range(gw):
                            sk = gw * g + j
                            nc.tensor.matmul(ov,
                                             v_all[:, sk * lf + h * 128:
                                                   sk * lf + (h + 1) * 128],
                                             ex_u[:, sk * nw:(sk + 1) * nw],
                                             start=(sk == 0),
                                             stop=(sk == ns - 1))
                        take = (((g + 1) * len(ops)) // ngrp
                                - (g * len(ops)) // ngrp)
                        for _ in range(take):
                            ops[oi]()
                            oi += 1
                        if not ops:
                            # first chunk has no out-projection yet: standalone
                            # weight loads keep the PE activity window non-idle
                            # (clock warm) through the ACT-paced wait slivers
                            for _ in range(2):
                                nc.tensor.ldweights(ones_b)
                    if u == 0:
                        rec = emit_sm_recip()
                    nc.vector.tensor_mul(aT_sq[:, h * nw:(h + 1) * nw], ov, rec)
                    if u + 1 < nunits:
                        accb_tiles[u + 1] = fold_finish(fold_stack)
                    if h == hpc - 1:
                        prev_c = (sq, aT_sq)
                # drain the final chunk's out-projection
                psq, pat = prev_c
                for op in outproj_ops(psq, pat, range(nsub)):
                    op()
    return nc


# ---------------------------------------------------------------------------
# Host-side sharding + gather
# ---------------------------------------------------------------------------

_PERM_HEAD = np.concatenate([np.arange(0, HD, 2), np.arange(1, HD, 2)])


def _pack_rows(a):
    """[n*128, m] -> [128, n*m]: kd-blocks of 128 rows side by side along the
    free dim — the SBUF-resident layout, so device DMAs are contiguous."""
    n = a.shape[0] // 128
    return np.ascontiguousarray(
        a.reshape(n, 128, a.shape[1]).transpose(1, 0, 2).reshape(128, -1))


def _unpack_y(yp, s, d):
    """[128, (s//128)*d] -> [s, d] (inverse of the device's packed store)."""
    n = s // 128
    return yp.reshape(128, n, d).transpose(1, 0, 2).reshape(s, d)


def _prep_in_maps(x, wq, wk, wv, wo, pos_cos, pos_sin, s=S, d=D, hpc=HPC):
    lf = hpc * HD
    h_total = d // HD
    groups = h_total // hpc
    # permute q/k feature rows within each head: even pairs first, then odd
    wq_p = wq.reshape(h_total, HD, d)[:, _PERM_HEAD, :].reshape(d, d)
    wk_p = wk.reshape(h_total, HD, d)[:, _PERM_HEAD, :].reshape(d, d)
    wqT_full = np.ascontiguousarray(wq_p.T).astype(NP_BF16)
    wkT_full = np.ascontiguousarray(wk_p.T).astype(NP_BF16)
    wvT_full = np.ascontiguousarray(wv.T).astype(NP_BF16)
    woT_full = np.ascontiguousarray(wo.T).astype(NP_BF16)
    cs_half = np.ascontiguousarray(pos_cos[0].T).astype(np.float32)  # [64, S]
    sn_half = np.ascontiguousarray(pos_sin[0].T).astype(np.float32)
    csd = np.concatenate([cs_half, cs_half], axis=0).astype(NP_BF16)
    snd = np.concatenate([sn_half, -sn_half], axis=0).astype(NP_BF16)
    in_maps = []
    n_batches = x.shape[0]
    # x packed sq-outermost: [128, sq][kd][nw] so each query chunk of every
    # contraction block is one contiguous device DMA
    kd_n = d // 128
    nw = 512 if s >= 512 else s
    nsq = s // nw

    def pack_x(xb):
        xt = np.ascontiguousarray(xb.T).astype(NP_BF16)  # [d, s]
        return np.ascontiguousarray(
            xt.reshape(kd_n, 128, nsq, nw).transpose(1, 2, 0, 3)
            .reshape(128, kd_n * s))

    xP = [pack_x(x[b]) for b in range(n_batches)]
    for c in range(n_batches * groups):
        b, g = divmod(c, groups)
        in_maps.append({
            "xT": xP[b],
            "wqT": _pack_rows(wqT_full[:, g * lf:(g + 1) * lf]),
            "wkT": _pack_rows(wkT_full[:, g * lf:(g + 1) * lf]),
            "wvT": _pack_rows(wvT_full[:, g * lf:(g + 1) * lf]),
            "woT": _pack_rows(woT_full[g * lf:(g + 1) * lf, :]),
            "csd": csd,
            "snd": snd,
        })
    return in_maps


_NC_CACHE = {}


def _get_nc(s=S, d=D, hpc=HPC):
    key = (s, d, hpc)
    if key not in _NC_CACHE:
        _NC_CACHE[key] = build_nc(s, d, hpc)
    return _NC_CACHE[key]


def _np_rope(t, cos, sin):
    b, ss, hh, hd = t.shape
    tr = t.reshape(b, ss, hh, hd // 2, 2)
    te, to = tr[..., 0], tr[..., 1]
    c = cos[:, :, None, :]
    s = sin[:, :, None, :]
    return np.stack([te * c - to * s, te * s + to * c], axis=-1).reshape(b, ss, hh, hd)


def _score_sample_max(x, wq, wk, pos_cos, pos_sin):
    """Sampled estimate of max |score|; the device softmax skips the max
    subtraction, which is only safe when scores stay well under exp's fp32
    range."""
    ss = x[:, :: max(1, x.shape[1] // 32), :][:, :32]
    pos_idx = np.arange(x.shape[1])[:: max(1, x.shape[1] // 32)][:32]
    h = x.shape[2] // HD
    q = (ss @ wq.T).reshape(ss.shape[0], -1, h, HD)
    k = (ss @ wk.T).reshape(ss.shape[0], -1, h, HD)
    c = pos_cos[:, pos_idx]
    sn = pos_sin[:, pos_idx]
    q = _np_rope(q, c, sn)
    k = _np_rope(k, c, sn)
    sc = np.einsum('bqhd,bkhd->bhqk', q, k) / math.sqrt(HD)
    return float(np.abs(sc).max())


def _np_fallback(x, wq, wk, wv, wo, pos_cos, pos_sin):
    out = np.empty_like(x)
    h = x.shape[2] // HD
    for b in range(x.shape[0]):
        q = _np_rope((x[b:b + 1] @ wq.T).reshape(1, -1, h, HD), pos_cos, pos_sin)
        k = _np_rope((x[b:b + 1] @ wk.T).reshape(1, -1, h, HD), pos_cos, pos_sin)
        v = (x[b:b + 1] @ wv.T).reshape(1, -1, h, HD)
        sc = np.einsum('bqhd,bkhd->bhqk', q, k) / math.sqrt(HD)
        sc -= sc.max(axis=-1, keepdims=True)
        e = np.exp(sc, dtype=np.float32)
        p = e / e.sum(axis=-1, keepdims=True)
        out[b] = (np.einsum('bhqk,bkhd->bqhd', p, v).reshape(1, x.shape[1], -1)
                  @ wo.T)[0]
    return out


def kernel(x, wq, wk, wv, wo, pos_cos, pos_sin):
    x = np.asarray(x, dtype=np.float32)
    wq, wk, wv, wo = (np.asarray(a, dtype=np.float32) for a in (wq, wk, wv, wo))
    pos_cos = np.asarray(pos_cos, dtype=np.float32)
    pos_sin = np.asarray(pos_sin, dtype=np.float32)
    # the device softmax skips max subtraction (safe for scores ~ N(0,1));
    # if the inputs are scaled such that exp would overflow, fall back to a
    # correct (slower) host path rather than returning inf/NaN
    if 4.0 * _score_sample_max(x, wq, wk, pos_cos, pos_sin) > 80.0:
        return _np_fallback(x, wq, wk, wv, wo, pos_cos, pos_sin)
    in_maps = _prep_in_maps(x, wq, wk, wv, wo, pos_cos, pos_sin)
    nc = _get_nc()
    res = run_bass_kernel_spmd(nc, in_maps, core_ids=list(range(N_CORES)))
    out = np.empty((B, S, D), dtype=np.float32)
    for b in range(B):
        acc = _unpack_y(res.results[b * GROUPS]["y"].astype(np.float32), S, D)
        for g in range(1, GROUPS):
            acc = acc + _unpack_y(
                res.results[b * GROUPS + g]["y"].astype(np.float32), S, D)
        out[b] = acc
    return out
